# revision 5
# baseline (speedup 1.0000x reference)
"""Block-diagonal linear kernel for Trainium2 (8 NeuronCores, data-parallel).

Problem: x [16384, 3072] f32, viewed as [B, C=1024, 3]; each 3-vector gets its
own Linear(3,3): y[b,c,o] = sum_i x[b,c,i]*W[c,o,i] + bias[c,o].

Sharding: data-parallel on the batch dim across 8 cores (2048 rows/core).
W and bias are tiny and replicated (pre-reshaped host-side into per-(o,i)
"weight image" rows of length C for broadcast into SBUF partitions).

Kernel strategy (v1): contiguous row DMA (batch rows on partitions), compute
with stride-3 free-dim slices on DVE: per output slot o, 3 muls + 3 adds
against weight images replicated across partitions.
"""

import numpy as np

import concourse.bass as bass
import concourse.bacc as bacc
import concourse.mybir as mybir
from concourse import bass_utils
from concourse.tile import TileContext

N_CORES = 8
B_FULL = 16384
F = 3072
C = F // 3  # 1024
B_CORE = B_FULL // N_CORES  # 2048
P = 128  # SBUF partitions
ROWS_PER_TILE = 128
N_TILES = B_CORE // ROWS_PER_TILE  # 16

FP32 = mybir.dt.float32


def build_bass():
    nc = bacc.Bacc("TRN2", num_devices=N_CORES)
    x = nc.dram_tensor("x", [B_CORE, F], FP32, kind="ExternalInput")
    # wb[p, o*3*C + i*C + c] = W[c, o, i]; wb[p, 9*C + o*C + c] = bias[c, o]
    # (pre-replicated across the 128 partitions host-side)
    wb = nc.dram_tensor("wb", [P, 12 * C], FP32, kind="ExternalInput")
    y = nc.dram_tensor("y", [B_CORE, F], FP32, kind="ExternalOutput")

    with TileContext(nc) as tc:
        with (
            tc.tile_pool(name="wpool", bufs=1) as wpool,
            tc.tile_pool(name="xpool", bufs=3) as xpool,
            tc.tile_pool(name="ypool", bufs=3) as ypool,
            tc.tile_pool(name="tpool", bufs=6) as tpool,
        ):
            wb_sb = wpool.tile([P, 12 * C], FP32)
            nc.sync.dma_start(out=wb_sb[:, :], in_=wb.ap()[:, :])
            w_sb = wb_sb[:, : 9 * C]
            b_sb = wb_sb[:, 9 * C :]
            # Touch wb_sb on DVE so its DMA-wait lands here; later DVE ops
            # then need only one wait slot (gen3 TT has a single wait slot).
            probe = wpool.tile([P, 1], FP32)
            nc.vector.tensor_copy(out=probe[:, :], in_=wb_sb[:, :1])

            for t in range(N_TILES):
                r0 = t * ROWS_PER_TILE
                xt = xpool.tile([P, F], FP32, tag="x")
                yt = ypool.tile([P, F], FP32, tag="y")
                nc.sync.dma_start(out=xt[:, :], in_=x.ap()[r0 : r0 + ROWS_PER_TILE, :])

                # Strided views [P, C] with step 3 at offset i (features c*3+i)
                x3 = xt[:, :].rearrange("p (c three) -> p c three", three=3)
                y3 = yt[:, :].rearrange("p (c three) -> p c three", three=3)

                for o in range(3):
                    acc = tpool.tile([P, C], FP32, tag="acc")
                    tmp = tpool.tile([P, C], FP32, tag="tmp")
                    w_o = lambda i: w_sb[:, (o * 3 + i) * C : (o * 3 + i + 1) * C]
                    nc.vector.tensor_mul(acc[:, :], x3[:, :, 0], w_o(0))
                    nc.vector.tensor_mul(tmp[:, :], x3[:, :, 1], w_o(1))
                    nc.vector.tensor_add(acc[:, :], acc[:, :], tmp[:, :])
                    nc.vector.tensor_mul(tmp[:, :], x3[:, :, 2], w_o(2))
                    nc.vector.tensor_add(acc[:, :], acc[:, :], tmp[:, :])
                    nc.vector.tensor_add(
                        y3[:, :, o], acc[:, :], b_sb[:, o * C : (o + 1) * C]
                    )

                nc.sync.dma_start(out=y.ap()[r0 : r0 + ROWS_PER_TILE, :], in_=yt[:, :])

    nc.compile()
    return nc


def _prep_small(W, b):
    wimg = W.transpose(1, 2, 0).reshape(9 * C).astype(np.float32)  # [o,i,c]
    bimg = b.T.reshape(3 * C).astype(np.float32)  # [o,c]
    flat = np.concatenate([wimg, bimg])  # [12*C]
    return np.ascontiguousarray(np.broadcast_to(flat, (P, 12 * C)))


def run(x, W, b, trace=False, **run_kwargs):
    nc = build_bass()
    wb = _prep_small(np.asarray(W), np.asarray(b))
    x = np.asarray(x, dtype=np.float32)
    in_maps = [
        {
            "x": np.ascontiguousarray(x[k * B_CORE : (k + 1) * B_CORE]),
            "wb": wb,
        }
        for k in range(N_CORES)
    ]
    res = bass_utils.run_bass_kernel_spmd(
        nc, in_maps, core_ids=list(range(N_CORES)), trace=trace, **run_kwargs
    )
    y = np.concatenate([r["y"] for r in res.results], axis=0)
    return y, res


def kernel(x, W, b):
    y, _ = run(x, W, b, trace=False)
    return y


# revision 6
# speedup vs baseline: 1.0231x; 1.0231x over previous
"""Block-linear v3: fp16 DVE compute + ScalarE deint/interleave, fused groups.

Per group of GT=2 x-tiles (256 batch rows):
  1. One SWDGE cast-DMA in: x f32 DRAM -> x16 fp16 SBUF [P, GT, F]
     (partition p holds rows r0+p and r0+128+p)
  2. ScalarE: 3 deinterleave copies (FD GT*C, strided): x16[., t, 3c+i] -> xd[i]
  3. DVE fp16 2x: per o: 3 muls + 3 adds (weights via step-0 broadcast APs)
  4. ScalarE: 3 interleave copies: yd[o] -> y16[., t, 3c+o]
  5. One SWDGE cast-DMA out: y16 fp16 -> y f32 DRAM
"""

import numpy as np

import concourse.bacc as bacc
import concourse.mybir as mybir
from concourse import bass_utils
from concourse.tile import TileContext

N_CORES = 8
B_FULL = 16384
F = 3072
C = F // 3  # 1024
B_CORE = B_FULL // N_CORES  # 2048
P = 128
GROUPS = [1, 1] + [2] * 6 + [1, 1]  # tiles per fused group (sum = 16)
FP32 = mybir.dt.float32
FP16 = mybir.dt.float16


def build_bass():
    nc = bacc.Bacc("TRN2", num_devices=N_CORES)
    x = nc.dram_tensor("x", [B_CORE, F], FP32, kind="ExternalInput")
    wb = nc.dram_tensor("wb16", [P, 12 * C], FP16, kind="ExternalInput")
    y = nc.dram_tensor("y", [B_CORE, F], FP32, kind="ExternalOutput")

    with TileContext(nc) as tc:
        with (
            tc.tile_pool(name="wpool", bufs=1) as wpool,
            tc.tile_pool(name="xpool", bufs=2) as xpool,
            tc.tile_pool(name="ypool", bufs=3) as ypool,
            tc.tile_pool(name="xdpool", bufs=3) as xdpool,
            tc.tile_pool(name="ydpool", bufs=3) as ydpool,
            tc.tile_pool(name="tpool", bufs=3) as tpool,
        ):
            wb_sb = wpool.tile([P, 12 * C], FP16)
            nc.sync.dma_start(out=wb_sb[:, :], in_=wb.ap()[:, :])
            wimg = lambda o, i, gt: (
                wb_sb[:, (o * 3 + i) * C : (o * 3 + i + 1) * C]
                .unsqueeze(1)
                .broadcast_to([P, gt, C])
            )
            bimg = lambda o, gt: (
                wb_sb[:, (9 + o) * C : (9 + o + 1) * C]
                .unsqueeze(1)
                .broadcast_to([P, gt, C])
            )
            probe = wpool.tile([P, 1], FP16)
            nc.vector.tensor_copy(out=probe[:, :], in_=wb_sb[:, :1])
            probe2 = wpool.tile([P, 1], FP16)
            nc.scalar.copy(probe2[:, :], wb_sb[:, :1])

            tile0 = 0
            for g, gt in enumerate(GROUPS):
                r0 = tile0 * P
                tile0 += gt
                x16 = xpool.tile([P, gt * F], FP16, tag="x", name=f"x16_{g}")
                y16 = ypool.tile([P, gt * F], FP16, tag="y", name=f"y16_{g}")
                xdram = x.ap()[r0 : r0 + gt * P, :].rearrange(
                    "(t p) f -> p t f", p=P
                )
                ydram = y.ap()[r0 : r0 + gt * P, :].rearrange(
                    "(t p) f -> p t f", p=P
                )
                # cast-DMA in (SWDGE): [p, t, f]
                nc.gpsimd.dma_start(
                    out=x16[:, :].rearrange("p (t f) -> p t f", f=F),
                    in_=xdram,
                )
                # [p, t, c, i] view
                x4 = x16[:, :].rearrange(
                    "p (t c three) -> p t c three", t=gt, three=3
                )
                y4 = y16[:, :].rearrange(
                    "p (t c three) -> p t c three", t=gt, three=3
                )

                xd = [
                    xdpool.tile([P, gt * C], FP16, tag=f"xd{i}", name=f"xd{i}_{g}")
                    for i in range(3)
                ]
                for i in range(3):
                    nc.scalar.copy(
                        xd[i][:, :].rearrange("p (t c) -> p t c", c=C),
                        x4[:, :, :, i],
                    )

                for o in range(3):
                    acc = tpool.tile([P, gt * C], FP16, tag="acc", name=f"acc_{g}_{o}")
                    tmp = tpool.tile([P, gt * C], FP16, tag="tmp", name=f"tmp_{g}_{o}")
                    yd = ydpool.tile([P, gt * C], FP16, tag=f"yd{o}", name=f"yd{o}_{g}")
                    a3 = acc[:, :].rearrange("p (t c) -> p t c", c=C)
                    t3 = tmp[:, :].rearrange("p (t c) -> p t c", c=C)
                    yd3 = yd[:, :].rearrange("p (t c) -> p t c", c=C)
                    xd3 = [d[:, :].rearrange("p (t c) -> p t c", c=C) for d in xd]
                    nc.vector.tensor_mul(a3, xd3[0], wimg(o, 0, gt))
                    nc.vector.tensor_mul(t3, xd3[1], wimg(o, 1, gt))
                    nc.vector.tensor_add(acc[:, :], acc[:, :], tmp[:, :])
                    nc.vector.tensor_mul(t3, xd3[2], wimg(o, 2, gt))
                    nc.vector.tensor_add(acc[:, :], acc[:, :], tmp[:, :])
                    nc.vector.tensor_add(yd3, a3, bimg(o, gt))
                    nc.scalar.copy(y4[:, :, :, o], yd3)

                # cast-DMA out (SWDGE)
                nc.gpsimd.dma_start(
                    out=ydram,
                    in_=y16[:, :].rearrange("p (t f) -> p t f", f=F),
                )

    nc.compile()
    return nc


def _prep_small(W, b):
    wimg = W.transpose(1, 2, 0).reshape(9 * C)
    bimg = b.T.reshape(3 * C)
    flat = np.concatenate([wimg, bimg]).astype(np.float16)
    return np.ascontiguousarray(np.broadcast_to(flat, (P, 12 * C)))


def run(x, W, b, trace=False, **run_kwargs):
    nc = build_bass()
    wb = _prep_small(np.asarray(W), np.asarray(b))
    x = np.asarray(x, dtype=np.float32)
    in_maps = [
        {"x": np.ascontiguousarray(x[k * B_CORE : (k + 1) * B_CORE]), "wb16": wb}
        for k in range(N_CORES)
    ]
    res = bass_utils.run_bass_kernel_spmd(
        nc, in_maps, core_ids=list(range(N_CORES)), trace=trace, **run_kwargs
    )
    y = np.concatenate([r["y"] for r in res.results], axis=0)
    return y, res


def kernel(x, W, b):
    y, _ = run(x, W, b, trace=False)
    return y
